# revision 26
# baseline (speedup 1.0000x reference)
"""CombinedEmbedding kernel for Trainium2 (8 NeuronCores, Bass/Tile).

Strategy
--------
Shard the sequence dim S=2048 across 8 cores (256 positions x 32 batches =
8192 tokens/core).  Sharding S (not B) makes the cross-batch
`counts_sum = building_counts.sum(axis=0)` reduction core-local.

The whole module is linear except the Time2Vec sine, so on the host we fold
the final projection into every table:
    g_n2v = node2vec_table @ proj_w[0:128]      [100000, 64]
    g_bt  = btype_table    @ proj_w[192:224]    [64, 64]
    g_et  = etype_table    @ proj_w[240:272]    [50, 64]
    g_eq  = equip_table    @ proj_w[272:304]    [100, 64]
On device each 128-token tile does 4 indirect-DMA row gathers of the
pre-projected tables (n2v/et/eq accumulate in the DMA engines via cce add),
the building gather is scaled by counts_sum (per-partition scalar) and added.
The sine path: the host supplies [x | population | 1] already transposed as
dense [10, 128] per-tile blocks; a small matmul forms the 56 pre-sine affine
values feature-major, fused sin+bias on the scalar engine, then two matmuls
accumulate sin-part + (x-linear, population, all-bias) parts into PSUM.
"""

import numpy as np

B, S, F, E = 32, 2048, 8, 8
N_CORES = 8
S_LOC = S // N_CORES          # 256
N_TOK = B * S_LOC             # 8192 tokens per core
P = 128                       # partitions / tokens per tile
N_TILES = N_TOK // P          # 64
XW = 10                       # rows per tile in xpbT layout: 8 x + pop + one
D_OUT = 64
N_SIN = F * (E - 1)           # 56
TWO_PI = float(np.float32(2 * np.pi))
MAGIC = 12582912.0            # 1.5 * 2**23: fp32 add rounds to nearest int
CLAMP = 0.4999998             # keep sin arg strictly inside [-pi, pi]

_CACHE = {}


def _build_nc():
    import concourse.bass as bass
    import concourse.mybir as mybir
    import concourse.tile as tile
    from concourse import bacc
    from contextlib import ExitStack

    f32 = mybir.dt.float32
    i32 = mybir.dt.int32

    nc = bacc.Bacc("TRN2", target_bir_lowering=False, debug=False)

    # DRAM I/O
    g_n2v = nc.dram_tensor("g_n2v", [100000, D_OUT], f32, kind="ExternalInput")
    g_bt = nc.dram_tensor("g_bt", [64, D_OUT], f32, kind="ExternalInput")
    g_et = nc.dram_tensor("g_et", [50, D_OUT], f32, kind="ExternalInput")
    g_eq = nc.dram_tensor("g_eq", [100, D_OUT], f32, kind="ExternalInput")
    wsin_d = nc.dram_tensor("wsin", [64 + N_SIN, D_OUT], f32, kind="ExternalInput")
    e2s_d = nc.dram_tensor("e2s", [F + 1, 64], f32, kind="ExternalInput")
    wxpb_d = nc.dram_tensor("wxpb", [XW, D_OUT], f32, kind="ExternalInput")
    xpb_d = nc.dram_tensor("xpbT", [N_TILES, XW, P], f32, kind="ExternalInput")
    idn_d = nc.dram_tensor("idn", [P, N_TILES], i32, kind="ExternalInput")
    idb_d = nc.dram_tensor("idb", [P, N_TILES], i32, kind="ExternalInput")
    ide_d = nc.dram_tensor("ide", [P, N_TILES], i32, kind="ExternalInput")
    idq_d = nc.dram_tensor("idq", [P, N_TILES], i32, kind="ExternalInput")
    cts_d = nc.dram_tensor("countsT", [S_LOC, B], f32, kind="ExternalInput")
    out_d = nc.dram_tensor("out", [N_TILES, P, D_OUT], f32, kind="ExternalOutput")

    with tile.TileContext(nc) as tc, ExitStack() as ctx:
        const = ctx.enter_context(tc.tile_pool(name="const", bufs=1))

        wsin = const.tile([64 + N_SIN, D_OUT], f32)
        nc.sync.dma_start(wsin[:], wsin_d.ap())
        e2s = const.tile([F + 1, 64], f32)
        nc.sync.dma_start(e2s[:], e2s_d.ap())
        wxpb = const.tile([XW, D_OUT], f32)
        nc.sync.dma_start(wxpb[:], wxpb_d.ap())

        zcol = const.tile([P, 1], f32)
        nc.vector.memset(zcol[:], 0.0)

        idn = const.tile([P, N_TILES], i32)
        nc.sync.dma_start(idn[:], idn_d.ap())
        idb = const.tile([P, N_TILES], i32)
        nc.sync.dma_start(idb[:], idb_d.ap())
        ide = const.tile([P, N_TILES], i32)
        nc.sync.dma_start(ide[:], ide_d.ap())
        idq = const.tile([P, N_TILES], i32)
        nc.sync.dma_start(idq[:], idq_d.ap())

        # counts_sum over batch: load [s_loc, b] transposed slice, reduce free dim
        cts0 = const.tile([P, B], f32)
        nc.sync.dma_start(cts0[:], cts_d.ap()[0:P, :])
        cts1 = const.tile([P, B], f32)
        nc.sync.dma_start(cts1[:], cts_d.ap()[P : 2 * P, :])
        csum = [
            const.tile([P, 1], f32, name="csum0"),
            const.tile([P, 1], f32, name="csum1"),
        ]
        nc.vector.tensor_reduce(
            csum[0][:], cts0[:], axis=mybir.AxisListType.X, op=mybir.AluOpType.add
        )
        nc.vector.tensor_reduce(
            csum[1][:], cts1[:], axis=mybir.AxisListType.X, op=mybir.AluOpType.add
        )

        xt_pool = ctx.enter_context(tc.tile_pool(name="xt", bufs=4))
        xa_pool = ctx.enter_context(tc.tile_pool(name="xa", bufs=2, space="PSUM"))
        u_pool = ctx.enter_context(tc.tile_pool(name="u", bufs=2))
        nd_pool = ctx.enter_context(tc.tile_pool(name="nd", bufs=2))
        t2v_pool = ctx.enter_context(tc.tile_pool(name="t2v", bufs=2))
        op_pool = ctx.enter_context(tc.tile_pool(name="op", bufs=4, space="PSUM"))
        acc_pool = ctx.enter_context(tc.tile_pool(name="acc", bufs=8))
        bt_pool = ctx.enter_context(tc.tile_pool(name="bt", bufs=8))
        ob_pool = ctx.enter_context(tc.tile_pool(name="ob", bufs=8))

        for tp in range(N_TILES // 2):
            # ---- time2vec sine path, two tiles packed per PSUM block ----
            # e2s/wsin are pre-scaled by 1/2pi on the host and the t2v bias
            # rides the ones-row, so xa = (x*W + b)/2pi.  Range reduction:
            #   u  = xa + MAGIC          (rounds to k = nearest int, exactly)
            #   nd = (u - MAGIC) - xa    = k - xa  in [-0.5, 0.5]
            #   sin(2pi * nd) = -sin(x*W + b)      (negation folded into wsin)
            xts = []
            xa = xa_pool.tile([P, P], f32)
            for h in range(2):
                xt = xt_pool.tile([XW, P], f32, name=f"xt{h}_{tp}")
                nc.sync.dma_start(xt[:], xpb_d.ap()[2 * tp + h])
                nc.tensor.matmul(
                    xa[64 * h : 64 * h + 64, :],
                    lhsT=e2s[:],
                    rhs=xt[0 : F + 1, :],
                    start=True,
                    stop=True,
                )
                xts.append(xt)
            u = u_pool.tile([P, P], f32)
            nc.vector.tensor_scalar(u[:], xa[:], MAGIC, None, op0=mybir.AluOpType.add)
            nd = nd_pool.tile([P, P], f32)
            nc.vector.scalar_tensor_tensor(
                nd[:],
                u[:],
                MAGIC,
                xa[:],
                op0=mybir.AluOpType.subtract,
                op1=mybir.AluOpType.subtract,
            )
            nc.vector.tensor_scalar(
                nd[:], nd[:], -CLAMP, CLAMP,
                op0=mybir.AluOpType.max, op1=mybir.AluOpType.min,
            )
            t2v = t2v_pool.tile([P, P], f32)
            nc.scalar.activation(
                t2v[:], nd[:], mybir.ActivationFunctionType.Sin,
                bias=zcol[:, 0:1], scale=TWO_PI,
            )

            for h in range(2):
                t = 2 * tp + h
                opsum = op_pool.tile([P, D_OUT], f32)
                nc.tensor.matmul(
                    opsum[:],
                    lhsT=t2v[64 * h : 64 * h + N_SIN, :],
                    rhs=wsin[64 * h : 64 * h + N_SIN, :],
                    start=True, stop=False,
                )
                nc.tensor.matmul(
                    opsum[:], lhsT=xts[h][:], rhs=wxpb[:], start=False, stop=True
                )

                # ---- pre-projected table gathers (DMA-accumulated) ----
                acc = acc_pool.tile([P, D_OUT], f32)
                nc.gpsimd.indirect_dma_start(
                    out=acc[:],
                    out_offset=None,
                    in_=g_n2v.ap(),
                    in_offset=bass.IndirectOffsetOnAxis(ap=idn[:, t : t + 1], axis=0),
                )
                nc.gpsimd.indirect_dma_start(
                    out=acc[:],
                    out_offset=None,
                    in_=g_et.ap(),
                    in_offset=bass.IndirectOffsetOnAxis(ap=ide[:, t : t + 1], axis=0),
                    compute_op=mybir.AluOpType.add,
                )
                nc.gpsimd.indirect_dma_start(
                    out=acc[:],
                    out_offset=None,
                    in_=g_eq.ap(),
                    in_offset=bass.IndirectOffsetOnAxis(ap=idq[:, t : t + 1], axis=0),
                    compute_op=mybir.AluOpType.add,
                )
                btt = bt_pool.tile([P, D_OUT], f32)
                nc.gpsimd.indirect_dma_start(
                    out=btt[:],
                    out_offset=None,
                    in_=g_bt.ap(),
                    in_offset=bass.IndirectOffsetOnAxis(ap=idb[:, t : t + 1], axis=0),
                )

                # ob = (btt * csum) + acc, then += opsum
                ob = ob_pool.tile([P, D_OUT], f32)
                nc.vector.scalar_tensor_tensor(
                    ob[:], btt[:], csum[t % 2][:, 0:1], acc[:],
                    op0=mybir.AluOpType.mult, op1=mybir.AluOpType.add,
                )
                nc.vector.tensor_add(ob[:], ob[:], opsum[:])
                nc.sync.dma_start(out_d.ap()[t], ob[:])

    nc.compile()
    return nc


def _host_prep(inputs):
    """Fold proj into tables; build per-core input maps."""
    f = lambda k: np.asarray(inputs[k], dtype=np.float32)
    ids = lambda k: np.asarray(inputs[k], dtype=np.int32)

    n2v = f("node2vec_table")
    t2v_w = f("t2v_weight")
    t2v_b = f("t2v_bias")
    btype = f("btype_table")
    pop_w = f("pop_w")
    pop_b = f("pop_b")
    etype = f("etype_table")
    equip = f("equip_table")
    proj_w = f("proj_w")
    proj_b = f("proj_b")

    W_sp = proj_w[0:128]
    W_t2v = proj_w[128:192]          # rows 128 + f*8 + e
    W_bt = proj_w[192:224]
    W_pop = proj_w[224:240]
    W_et = proj_w[240:272]
    W_eq = proj_w[272:304]

    g_n2v = np.ascontiguousarray(n2v @ W_sp)
    g_bt = np.ascontiguousarray(btype @ W_bt)
    g_et = np.ascontiguousarray(etype @ W_et)
    g_eq = np.ascontiguousarray(equip @ W_eq)

    # sine-path constants (feature order r = (e-1)*8 + f for e in 1..7).
    # e2s carries weights AND bias (row F rides the ones input), pre-scaled
    # by 1/2pi; wsin is negated because the device computes -sin(x*W + b).
    inv2pi = 1.0 / (2 * np.pi)
    wsin = np.zeros((64 + N_SIN, D_OUT), np.float32)
    e2s = np.zeros((F + 1, 64), np.float32)
    for e in range(1, E):
        for fi in range(F):
            r = (e - 1) * F + fi
            wsin[r] = wsin[64 + r] = -W_t2v[fi * E + e]
            e2s[fi, r] = t2v_w[fi, e] * inv2pi
            e2s[F, r] = t2v_b[fi, e] * inv2pi

    # x-linear (e=0), constant terms (ones row), population
    wxpb = np.zeros((XW, D_OUT), np.float32)
    for fi in range(F):
        wxpb[fi] = t2v_w[fi, 0] * W_t2v[fi * E + 0]
    wxpb[8] = (
        sum(t2v_b[fi, 0] * W_t2v[fi * E + 0] for fi in range(F))
        + pop_b @ W_pop
        + proj_b
    )
    wxpb[9] = (pop_w @ W_pop)[0]

    tf = f("time_features")          # [B, S, F]
    pop = f("population")[..., 0]    # [B, S]
    cnt = f("building_counts")[..., 0]
    id_n = ids("neighborhood_ids")[..., 0]
    id_b = ids("building_type_ids")[..., 0]
    id_e = ids("event_type_ids")[..., 0]
    id_q = ids("equipment_ids")[..., 0]

    shared = {
        "g_n2v": g_n2v, "g_bt": g_bt, "g_et": g_et, "g_eq": g_eq,
        "wsin": wsin, "e2s": e2s, "wxpb": wxpb,
    }
    in_maps = []
    for c in range(N_CORES):
        sl = slice(c * S_LOC, (c + 1) * S_LOC)
        x_fl = tf[:, sl, :].reshape(N_TOK, F)
        p_fl = pop[:, sl].reshape(N_TOK)
        xpb = np.empty((N_TILES, XW, P), np.float32)
        xpb[:, 0:F, :] = x_fl.reshape(N_TILES, P, F).transpose(0, 2, 1)
        xpb[:, F, :] = 1.0                         # ones row (bias + consts)
        xpb[:, F + 1, :] = p_fl.reshape(N_TILES, P)  # population row

        def idt(a):  # [p, tile] layout
            return np.ascontiguousarray(a[:, sl].reshape(N_TOK).reshape(N_TILES, P).T)

        m = dict(shared)
        m.update(
            xpbT=xpb,
            idn=idt(id_n), idb=idt(id_b), ide=idt(id_e), idq=idt(id_q),
            countsT=np.ascontiguousarray(cnt[:, sl].T),
        )
        in_maps.append(m)
    return in_maps


def kernel(**inputs):
    from concourse.bass_utils import run_bass_kernel_spmd

    if "nc" not in _CACHE:
        _CACHE["nc"] = _build_nc()
    nc = _CACHE["nc"]

    in_maps = _host_prep(inputs)
    res = run_bass_kernel_spmd(nc, in_maps, core_ids=list(range(N_CORES)))

    out = np.empty((B, S, D_OUT), np.float32)
    for c in range(N_CORES):
        o = res.results[c]["out"].reshape(N_TOK, D_OUT).reshape(B, S_LOC, D_OUT)
        out[:, c * S_LOC : (c + 1) * S_LOC, :] = o
    return out
